# revision 13
# baseline (speedup 1.0000x reference)
"""Converse2D-Up (FFT deconvolution upsampler) as a Bass/Tile kernel for TRN2.

v3: fp16 + FX subtracts fused into stage C's PSUM accumulation.

Measured HW facts driving this structure (see ldw_bench.py):
- PE streams 0.42 ns/col (2.4 GHz); LDWEIGHTS hides behind the previous
  matmul when stationary K>=64ish; K<=8 stationaries cost ~2x per col.
- DVE/Pool tensor ops are free-dim BYTE-bound (~0.39 ns/B) regardless of
  partition count, so [4, x] lo ops cost the same as [128, x] ones and
  every elementwise byte matters.

Per image:
  A:  r1[y, u_ri] = x^T @ gt                                (1 mm, N=264)
  B:  Y[u,(Yr v|Yi v)] PSUM, hi 128 + lo 4 rows             (4 mms, N=134)
  FX: products only (no combine):
      DVE : tatb[128,(g,h,f,v)]  = [khi_a|khi_b] (.) Y-bcast   (2 mults)
      Pool: tatbl[4,(g,h,f,v)]   = [klo_a|klo_b] (.) Ylo-bcast (2 mults)
      DMA : tatbl[4,(q,c)] -> tatbl16[16=(u',q),268]  (reshape, 16 desc)
  C:  per 256-col block [Tr|Ti]_p: 5 accums with sign-folded movings
        P1@[CT|ST] + P2@-[CT|ST] + P3@[-ST|CT] + P4'@[ST|-CT]
        + tatbl16@cstlo16                                   (20 mms, N=256)
  D:  pD[x,(p,y)] += Tr_p @ RC + Ti_p @ RS                  (8 mms, N=128)
  gelu + (dx, n, dy)-ordered store: one ACT + one DMA per image.

Sharding: 8 channels/core x 4 batch images; all spectra host-precomputed.
"""

import os

import numpy as np

import concourse.bass as bass
import concourse.mybir as mybir
import concourse.tile as tile
from concourse import bacc
from concourse.bass_utils import run_bass_kernel_spmd

F32 = mybir.dt.float32
F16 = mybir.dt.float16
AF = mybir.ActivationFunctionType

SCALE = 2
PAD = 2
EPS = 1e-5
N0 = 128           # input spatial size
NP = N0 + 2 * PAD  # 132 padded
NU = NP * SCALE    # 264 upsampled
NV = NP // 2 + 1   # 67 unique spectral columns
B = 4
C = 64
NCORES = 8
CPC = C // NCORES  # 8 channels per core
NIMG = B * CPC     # 32 images per core
PH4 = 4 * NV       # 268

LAST_EXEC_NS = None  # set by kernel() when tracing is enabled


# --------------------------------------------------------------------------
# host-side constant precompute (weight/bias -> per-channel spectra)
# --------------------------------------------------------------------------

def _host_constants(weight, bias):
    w64 = np.asarray(weight, dtype=np.float64)
    b64 = np.asarray(bias, dtype=np.float64)

    k_h, k_w = w64.shape[-2:]
    otf = np.zeros((C, NU, NU), dtype=np.complex128)
    otf[:, :k_h, :k_w] = w64[0]
    otf = np.roll(otf, (-(k_h // 2), -(k_w // 2)), axis=(-2, -1))
    FB = np.fft.fftn(otf, axes=(-2, -1))                      # (C,264,264)

    biaseps = 1.0 / (1.0 + np.exp(-(b64.reshape(C) - 9.0))) + EPS
    be = biaseps[:, None, None]

    u = np.arange(NU)
    Dr = 1 + np.exp(-2j * np.pi * u / NU)
    D = Dr[:, None] * Dr[None, :]

    Gh = np.conj(FB) + be * D[None]
    FBG = FB * Gh

    def quadmean(A):
        return 0.25 * (A[:, :NP, :NP] + A[:, NP:, :NP]
                       + A[:, :NP, NP:] + A[:, NP:, NP:])

    M1 = quadmean(FBG)
    invW = quadmean(np.abs(FB) ** 2)
    M2 = M1 / (invW + be)
    H = (Gh - np.conj(FB) * np.tile(M2, (1, SCALE, SCALE))) / be

    hr = np.fft.ifft2(H, axes=(-2, -1)).real
    kdd = np.empty((C, 2, 2, NP, NV), dtype=np.complex128)
    for dx in range(2):
        for dy in range(2):
            kh = np.fft.fft2(hr[:, dx::2, dy::2], axes=(-2, -1))
            kdd[:, dx, dy] = kh[:, :, :NV]

    # K tiles: [u, (f, v)] per real/imag, phases f = 2*dx+dy
    kr = np.empty((C, NP, PH4), dtype=np.float64)
    ki = np.empty((C, NP, PH4), dtype=np.float64)
    for dx in range(2):
        for dy in range(2):
            p = dx * 2 + dy
            kr[:, :, p * NV:(p + 1) * NV] = kdd[:, dx, dy].real
            ki[:, :, p * NV:(p + 1) * NV] = kdd[:, dx, dy].imag

    # khi_a = [Kr | Ki], khi_b = [Ki | -Kr].
    # Products: P1=KrYr P2=KiYi P3=KiYr P4'=-KrYi  (q=(g,h) in order)
    khi_a = np.concatenate([kr[:, :128], ki[:, :128]], axis=2)
    khi_b = np.concatenate([ki[:, :128], -kr[:, :128]], axis=2)
    klo_a = np.concatenate([kr[:, 128:], ki[:, 128:]], axis=2)
    klo_b = np.concatenate([ki[:, 128:], -kr[:, 128:]], axis=2)

    # forward matrix G = F132 @ P  (132x128 complex)
    P = np.zeros((NP, N0))
    for m in range(NP):
        P[m, (m - PAD) % N0] = 1.0
    F132 = np.exp(-2j * np.pi * np.outer(np.arange(NP), np.arange(NP)) / NP)
    G = F132 @ P
    gt = np.concatenate([G.real.T, G.imag.T], axis=1)          # (128, 264)

    # stage-B movings: [Gr | Gi | -Gi | Gr]  (128, 268)
    gbv = np.concatenate([gt[:, 0:NV], gt[:, NP:NP + NV],
                          -gt[:, NP:NP + NV], gt[:, 0:NV]], axis=1)

    # inverse rows i in [2,130) of iF132/132
    Ai = np.exp(2j * np.pi * np.outer(np.arange(2, 130), np.arange(NP)) / NP) / NP
    Cm, Sm = Ai.real, Ai.imag
    CT = Cm.T                                                  # (132,128)
    ST = Sm.T

    # C movings, sign-folded per product q:
    #   [Tr|Ti] = P1@[CT|ST] - P2@[CT|ST] + P3@[-ST|CT] - P4'@[-ST|CT]
    cs = np.concatenate([CT[:128], ST[:128]], axis=1)          # (128,256)
    sc = np.concatenate([-ST[:128], CT[:128]], axis=1)
    csthi4 = np.concatenate([cs, -cs, sc, -sc], axis=1)        # (128,1024)

    # lo: tatbl16 rows (u', q); cstlo16 row (u', q) = sign_q * block_q(u')
    cslo = np.concatenate([CT[128:], ST[128:]], axis=1)        # (4,256)
    sclo = np.concatenate([-ST[128:], CT[128:]], axis=1)
    # rows match the 2-DMA reshape: row = 8*(q>=2) + u'*2 + (q%2)
    # (HW-verified: dma [4,536]->[8,268] flattens partition-major)
    blocks = [cslo, -cslo, sclo, -sclo]                        # per q
    cstlo16 = np.empty((16, 256))
    for up in range(4):
        for q in range(4):
            cstlo16[(8 if q >= 2 else 0) + up * 2 + (q % 2)] = blocks[q][up]

    w_v = np.ones(NV)
    w_v[1:NV - 1] = 2.0
    RC = (Cm[:, :NV] * w_v[None, :]).T                          # (67,128)
    RS = (-Sm[:, :NV] * w_v[None, :]).T
    rcsd = np.concatenate([RC, RS], axis=1)                     # (67,256)

    f16 = lambda a: np.ascontiguousarray(a.astype(np.float16))
    return {
        "khi_a": f16(khi_a), "khi_b": f16(khi_b),
        "klo_a": f16(klo_a), "klo_b": f16(klo_b),
        "gt": f16(gt), "gbv": f16(gbv),
        "csthi4": f16(csthi4), "cstlo16": f16(cstlo16), "rcsd": f16(rcsd),
    }


# --------------------------------------------------------------------------
# device kernel
# --------------------------------------------------------------------------

def build_nc(n_chan=CPC, n_batch=B, gelu=True):
    act_fn = AF.Gelu if gelu else AF.Copy
    n_img = n_chan * n_batch
    nc = bacc.Bacc("TRN2", target_bir_lowering=False, debug=False,
                   enable_asserts=False)

    x_t = nc.dram_tensor("x", [n_img, N0, N0], F16, kind="ExternalInput")
    khi_a_t = nc.dram_tensor("khi_a", [n_chan, 128, 2 * PH4], F16,
                             kind="ExternalInput")
    khi_b_t = nc.dram_tensor("khi_b", [n_chan, 128, 2 * PH4], F16,
                             kind="ExternalInput")
    klo_a_t = nc.dram_tensor("klo_a", [n_chan, 4, 2 * PH4], F16,
                             kind="ExternalInput")
    klo_b_t = nc.dram_tensor("klo_b", [n_chan, 4, 2 * PH4], F16,
                             kind="ExternalInput")
    gt_t = nc.dram_tensor("gt", [128, 2 * NP], F16, kind="ExternalInput")
    gbv_t = nc.dram_tensor("gbv", [128, 4 * NV], F16, kind="ExternalInput")
    csthi4_t = nc.dram_tensor("csthi4", [128, 1024], F16, kind="ExternalInput")
    cstlo16_t = nc.dram_tensor("cstlo16", [16, 256], F16, kind="ExternalInput")
    rcsd_t = nc.dram_tensor("rcsd", [NV, 256], F16, kind="ExternalInput")
    out_t = nc.dram_tensor("out", [n_img, 2 * N0, 2 * N0], F32,
                           kind="ExternalOutput")

    with tile.TileContext(nc) as tc:
        with (
            tc.tile_pool(name="consts", bufs=1) as cpool,
            tc.tile_pool(name="kdd", bufs=2) as kpool,
            tc.tile_pool(name="xin", bufs=4) as xpool,
            tc.tile_pool(name="r1", bufs=2) as r1pool,
            tc.tile_pool(name="ysb", bufs=2) as ypool,
            tc.tile_pool(name="fxt", bufs=2) as fxtpool,
            tc.tile_pool(name="t1", bufs=2) as t1pool,
            tc.tile_pool(name="osb", bufs=2) as opool,
            tc.tile_pool(name="ppa", bufs=1, space="PSUM") as ppa_pool,
            tc.tile_pool(name="py", bufs=2, space="PSUM") as py_pool,
            tc.tile_pool(name="pt1", bufs=3, space="PSUM") as pt1_pool,
            tc.tile_pool(name="ppd", bufs=2, space="PSUM") as ppd_pool,
        ):
            gt = cpool.tile([128, 2 * NP], F16)
            nc.sync.dma_start(gt[:], gt_t[:])
            gbv = cpool.tile([128, 4 * NV], F16)
            nc.sync.dma_start(gbv[:], gbv_t[:])
            csthi4 = cpool.tile([128, 1024], F16)
            nc.sync.dma_start(csthi4[:], csthi4_t[:])
            cstlo16 = cpool.tile([16, 256], F16)
            nc.sync.dma_start(cstlo16[:], cstlo16_t[:])
            rcsd = cpool.tile([NV, 256], F16)
            nc.sync.dma_start(rcsd[:], rcsd_t[:])

            for ci in range(n_chan):
                khi_a = kpool.tile([128, 2 * PH4], F16, tag="khi_a")
                nc.sync.dma_start(khi_a[:], khi_a_t[ci])
                khi_b = kpool.tile([128, 2 * PH4], F16, tag="khi_b")
                nc.sync.dma_start(khi_b[:], khi_b_t[ci])
                klo_a = kpool.tile([4, 2 * PH4], F16, tag="klo_a")
                nc.sync.dma_start(klo_a[:], klo_a_t[ci])
                klo_b = kpool.tile([4, 2 * PH4], F16, tag="klo_b")
                nc.sync.dma_start(klo_b[:], klo_b_t[ci])

                for bi in range(n_batch):
                    img = ci * n_batch + bi

                    # ---- stage A: r1[y, u_ri] = x^T @ gt ----
                    x_tile = xpool.tile([N0, N0], F16, tag="x")
                    nc.sync.dma_start(x_tile[:], x_t[img])
                    pA = ppa_pool.tile([128, NU], F32, tag="pA")
                    nc.tensor.matmul(pA[:], x_tile[:], gt[:],
                                     start=True, stop=True)
                    r1 = r1pool.tile([128, NU], F16, tag="r1")
                    nc.scalar.activation(r1[:], pA[:], AF.Copy)

                    # ---- stage B: Y[u, (rv)] hi + lo into one bank ----
                    pY = py_pool.tile([128, 2 * 2 * NV], F32, tag="pY")
                    yT = pY[:, 0:2 * NV]
                    ylo = pY[0:4, 2 * NV:4 * NV]
                    nc.tensor.matmul(yT, r1[:, 0:128], gbv[:, 0:2 * NV],
                                     start=True, stop=False,
                                     skip_group_check=True)
                    nc.tensor.matmul(yT, r1[:, NP:NP + 128],
                                     gbv[:, 2 * NV:4 * NV],
                                     start=False, stop=True,
                                     skip_group_check=True)
                    nc.tensor.matmul(ylo, r1[:, 128:NP], gbv[:, 0:2 * NV],
                                     start=True, stop=False,
                                     skip_group_check=True)
                    nc.tensor.matmul(ylo, r1[:, NP + 128:2 * NP],
                                     gbv[:, 2 * NV:4 * NV],
                                     start=False, stop=True,
                                     skip_group_check=True)

                    # evict Y to fp16 SBUF in ONE activation: cols 0:134
                    # hold yT (128 rows), cols 134:268 hold ylo (rows 0:4;
                    # rows 4:128 there are junk and never read)
                    ysbx = ypool.tile([128, 4 * NV], F16, tag="ysbx")
                    nc.scalar.activation(ysbx[:], pY[:], AF.Copy)
                    ysb = ysbx[:, 0:2 * NV]
                    ylo_sb = ysbx[0:4, 2 * NV:4 * NV]

                    # ---- FX products (no combines) ----
                    ybv = (ysb.rearrange("p (h v) -> p h v", h=2)
                           [:, :, None, :].broadcast_to([128, 2, 4, NV]))
                    tatb = fxtpool.tile([128, 4 * PH4], F16, tag="tatb")
                    nc.vector.tensor_mul(
                        tatb[:, 0:2 * PH4].rearrange(
                            "p (h f v) -> p h f v", h=2, f=4),
                        khi_a[:].rearrange("p (h f v) -> p h f v", h=2, f=4),
                        ybv)
                    nc.vector.tensor_mul(
                        tatb[:, 2 * PH4:4 * PH4].rearrange(
                            "p (h f v) -> p h f v", h=2, f=4),
                        khi_b[:].rearrange("p (h f v) -> p h f v", h=2, f=4),
                        ybv)

                    ylbv = (ylo_sb.rearrange("p (h v) -> p h v", h=2)
                            [:, :, None, :].broadcast_to([4, 2, 4, NV]))
                    tatbl = fxtpool.tile([4, 4 * PH4], F16, tag="tatbl")
                    nc.gpsimd.tensor_mul(
                        tatbl[:, 0:2 * PH4].rearrange(
                            "p (h f v) -> p h f v", h=2, f=4),
                        klo_a[:].rearrange("p (h f v) -> p h f v", h=2, f=4),
                        ylbv)
                    nc.gpsimd.tensor_mul(
                        tatbl[:, 2 * PH4:4 * PH4].rearrange(
                            "p (h f v) -> p h f v", h=2, f=4),
                        klo_b[:].rearrange("p (h f v) -> p h f v", h=2, f=4),
                        ylbv)
                    # reshape [4,(q,c)] -> [16,268] for one lo accum
                    # (row = 8*(q>=2) + u'*2 + q%2; cstlo16 matches)
                    tatbl16 = fxtpool.tile([16, PH4], F16, tag="tatbl16")
                    nc.sync.dma_start(tatbl16[0:8, :], tatbl[:, 0:2 * PH4])
                    nc.sync.dma_start(tatbl16[8:16, :],
                                      tatbl[:, 2 * PH4:4 * PH4])

                    # ---- stage C: 5 sign-folded accums per 256 block ----
                    t1g = t1pool.tile([NV, 1024], F16, tag="t1g")
                    for half in range(2):
                        pT1 = pt1_pool.tile([NV, 512], F32, tag="pT1")
                        for k in range(2):
                            p = 2 * half + k
                            o = pT1[:, k * 256:(k + 1) * 256]
                            for q in range(4):
                                sl = slice(q * PH4 + p * NV,
                                           q * PH4 + (p + 1) * NV)
                                nc.tensor.matmul(
                                    o, tatb[:, sl],
                                    csthi4[:, q * 256:(q + 1) * 256],
                                    start=(q == 0), stop=False)
                            nc.tensor.matmul(
                                o, tatbl16[:, p * NV:(p + 1) * NV],
                                cstlo16[:], start=False, stop=True)
                        dst = t1g[:, half * 512:(half + 1) * 512]
                        if half == 0:
                            nc.vector.tensor_copy(dst, pT1[:])
                        else:
                            nc.scalar.activation(dst, pT1[:], AF.Copy)

                    # ---- stage D: pD[x, (p, y)], 2 accums per phase ----
                    pD = ppd_pool.tile([128, 512], F32, tag="pD")
                    for p in range(4):
                        o = pD[:, p * 128:(p + 1) * 128]
                        nc.tensor.matmul(o, t1g[:, p * 256:p * 256 + 128],
                                         rcsd[:, 0:128],
                                         start=True, stop=False)
                        nc.tensor.matmul(o, t1g[:, p * 256 + 128:(p + 1) * 256],
                                         rcsd[:, 128:256],
                                         start=False, stop=True)

                    # ---- gelu + (dx, n, dy)-ordered store ----
                    outt = opool.tile([128, 512], F32, tag="outt")
                    nc.scalar.activation(
                        outt[:].rearrange("p (dx n dy) -> p dx dy n",
                                          dx=2, n=128),
                        pD[:].rearrange("p (dx dy n) -> p dx dy n",
                                        dx=2, dy=2),
                        act_fn)
                    nc.scalar.dma_start(
                        out_t[img].rearrange("(m dx) (n dy) -> m (dx n dy)",
                                             dx=2, dy=2),
                        outt[:])

    nc.compile()
    return nc


# --------------------------------------------------------------------------
# public entry point: full inputs in, full output out
# --------------------------------------------------------------------------

def kernel(x, weight, bias):
    global LAST_EXEC_NS
    x16 = np.ascontiguousarray(np.asarray(x, dtype=np.float16))
    consts = _host_constants(weight, bias)

    nc = build_nc()

    in_maps = []
    for core in range(NCORES):
        c0 = core * CPC
        xs = np.ascontiguousarray(
            x16[:, c0:c0 + CPC].transpose(1, 0, 2, 3)).reshape(NIMG, N0, N0)
        in_maps.append({
            "x": xs,
            "khi_a": np.ascontiguousarray(consts["khi_a"][c0:c0 + CPC]),
            "khi_b": np.ascontiguousarray(consts["khi_b"][c0:c0 + CPC]),
            "klo_a": np.ascontiguousarray(consts["klo_a"][c0:c0 + CPC]),
            "klo_b": np.ascontiguousarray(consts["klo_b"][c0:c0 + CPC]),
            "gt": consts["gt"],
            "gbv": consts["gbv"],
            "csthi4": consts["csthi4"],
            "cstlo16": consts["cstlo16"],
            "rcsd": consts["rcsd"],
        })

    trace = os.environ.get("KERNEL_TRACE", "0") == "1"
    tmpdir = os.environ.get("KERNEL_TMPDIR") or None
    res = run_bass_kernel_spmd(nc, in_maps, list(range(NCORES)), trace=trace,
                               tmpdir=tmpdir)
    LAST_EXEC_NS = res.exec_time_ns

    out = np.empty((B, C, 2 * N0, 2 * N0), dtype=np.float32)
    for core in range(NCORES):
        c0 = core * CPC
        o = res.results[core]["out"].reshape(CPC, B, 2 * N0, 2 * N0)
        out[:, c0:c0 + CPC] = o.transpose(1, 0, 2, 3)
    return out


# revision 14
# speedup vs baseline: 1.0495x; 1.0495x over previous
"""Converse2D-Up (FFT deconvolution upsampler) as a Bass/Tile kernel for TRN2.

v3: fp16 + FX subtracts fused into stage C's PSUM accumulation.

Measured HW facts driving this structure (see ldw_bench.py):
- PE streams 0.42 ns/col (2.4 GHz); LDWEIGHTS hides behind the previous
  matmul when stationary K>=64ish; K<=8 stationaries cost ~2x per col.
- DVE/Pool tensor ops are free-dim BYTE-bound (~0.39 ns/B) regardless of
  partition count, so [4, x] lo ops cost the same as [128, x] ones and
  every elementwise byte matters.

Per image:
  A:  r1[y, u_ri] = x^T @ gt                                (1 mm, N=264)
  B:  Y[u,(Yr v|Yi v)] PSUM, hi 128 + lo 4 rows             (4 mms, N=134)
  FX: products only (no combine):
      DVE : tatb[128,(g,h,f,v)]  = [khi_a|khi_b] (.) Y-bcast   (2 mults)
      Pool: tatbl[4,(g,h,f,v)]   = [klo_a|klo_b] (.) Ylo-bcast (2 mults)
      DMA : tatbl[4,(q,c)] -> tatbl16[16=(u',q),268]  (reshape, 16 desc)
  C:  per 256-col block [Tr|Ti]_p: 5 accums with sign-folded movings
        P1@[CT|ST] + P2@-[CT|ST] + P3@[-ST|CT] + P4'@[ST|-CT]
        + tatbl16@cstlo16                                   (20 mms, N=256)
  D:  pD[x,(p,y)] += Tr_p @ RC + Ti_p @ RS                  (8 mms, N=128)
  gelu + (dx, n, dy)-ordered store: one ACT + one DMA per image.

Sharding: 8 channels/core x 4 batch images; all spectra host-precomputed.
"""

import os

import numpy as np

import concourse.bass as bass
import concourse.mybir as mybir
import concourse.tile as tile
from concourse import bacc
from concourse.bass_utils import run_bass_kernel_spmd

F32 = mybir.dt.float32
F16 = mybir.dt.float16
AF = mybir.ActivationFunctionType

SCALE = 2
PAD = 2
EPS = 1e-5
N0 = 128           # input spatial size
NP = N0 + 2 * PAD  # 132 padded
NU = NP * SCALE    # 264 upsampled
NV = NP // 2 + 1   # 67 unique spectral columns
B = 4
C = 64
NCORES = 8
CPC = C // NCORES  # 8 channels per core
NIMG = B * CPC     # 32 images per core
PH4 = 4 * NV       # 268

LAST_EXEC_NS = None  # set by kernel() when tracing is enabled


# --------------------------------------------------------------------------
# host-side constant precompute (weight/bias -> per-channel spectra)
# --------------------------------------------------------------------------

def _host_constants(weight, bias):
    w64 = np.asarray(weight, dtype=np.float64)
    b64 = np.asarray(bias, dtype=np.float64)

    k_h, k_w = w64.shape[-2:]
    otf = np.zeros((C, NU, NU), dtype=np.complex128)
    otf[:, :k_h, :k_w] = w64[0]
    otf = np.roll(otf, (-(k_h // 2), -(k_w // 2)), axis=(-2, -1))
    FB = np.fft.fftn(otf, axes=(-2, -1))                      # (C,264,264)

    biaseps = 1.0 / (1.0 + np.exp(-(b64.reshape(C) - 9.0))) + EPS
    be = biaseps[:, None, None]

    u = np.arange(NU)
    Dr = 1 + np.exp(-2j * np.pi * u / NU)
    D = Dr[:, None] * Dr[None, :]

    Gh = np.conj(FB) + be * D[None]
    FBG = FB * Gh

    def quadmean(A):
        return 0.25 * (A[:, :NP, :NP] + A[:, NP:, :NP]
                       + A[:, :NP, NP:] + A[:, NP:, NP:])

    M1 = quadmean(FBG)
    invW = quadmean(np.abs(FB) ** 2)
    M2 = M1 / (invW + be)
    H = (Gh - np.conj(FB) * np.tile(M2, (1, SCALE, SCALE))) / be

    hr = np.fft.ifft2(H, axes=(-2, -1)).real
    kdd = np.empty((C, 2, 2, NP, NV), dtype=np.complex128)
    for dx in range(2):
        for dy in range(2):
            kh = np.fft.fft2(hr[:, dx::2, dy::2], axes=(-2, -1))
            kdd[:, dx, dy] = kh[:, :, :NV]

    # K tiles: [u, (f, v)] per real/imag, phases f = 2*dx+dy
    kr = np.empty((C, NP, PH4), dtype=np.float64)
    ki = np.empty((C, NP, PH4), dtype=np.float64)
    for dx in range(2):
        for dy in range(2):
            p = dx * 2 + dy
            kr[:, :, p * NV:(p + 1) * NV] = kdd[:, dx, dy].real
            ki[:, :, p * NV:(p + 1) * NV] = kdd[:, dx, dy].imag

    # khi_a = [Kr | Ki], khi_b = [Ki | -Kr].
    # Products: P1=KrYr P2=KiYi P3=KiYr P4'=-KrYi  (q=(g,h) in order)
    khi_a = np.concatenate([kr[:, :128], ki[:, :128]], axis=2)
    khi_b = np.concatenate([ki[:, :128], -kr[:, :128]], axis=2)
    klo_a = np.concatenate([kr[:, 128:], ki[:, 128:]], axis=2)
    klo_b = np.concatenate([ki[:, 128:], -kr[:, 128:]], axis=2)

    # forward matrix G = F132 @ P  (132x128 complex)
    P = np.zeros((NP, N0))
    for m in range(NP):
        P[m, (m - PAD) % N0] = 1.0
    F132 = np.exp(-2j * np.pi * np.outer(np.arange(NP), np.arange(NP)) / NP)
    G = F132 @ P
    gt = np.concatenate([G.real.T, G.imag.T], axis=1)          # (128, 264)

    # stage-B movings: [Gr | Gi | -Gi | Gr]  (128, 268)
    gbv = np.concatenate([gt[:, 0:NV], gt[:, NP:NP + NV],
                          -gt[:, NP:NP + NV], gt[:, 0:NV]], axis=1)

    # inverse rows i in [2,130) of iF132/132
    Ai = np.exp(2j * np.pi * np.outer(np.arange(2, 130), np.arange(NP)) / NP) / NP
    Cm, Sm = Ai.real, Ai.imag
    CT = Cm.T                                                  # (132,128)
    ST = Sm.T

    # C movings, sign-folded per product q:
    #   [Tr|Ti] = P1@[CT|ST] - P2@[CT|ST] + P3@[-ST|CT] - P4'@[-ST|CT]
    cs = np.concatenate([CT[:128], ST[:128]], axis=1)          # (128,256)
    sc = np.concatenate([-ST[:128], CT[:128]], axis=1)
    csthi4 = np.concatenate([cs, -cs, sc, -sc], axis=1)        # (128,1024)

    # lo: tatbl16 rows (u', q); cstlo16 row (u', q) = sign_q * block_q(u')
    cslo = np.concatenate([CT[128:], ST[128:]], axis=1)        # (4,256)
    sclo = np.concatenate([-ST[128:], CT[128:]], axis=1)
    # rows match the 2-DMA reshape: row = 8*(q>=2) + u'*2 + (q%2)
    # (HW-verified: dma [4,536]->[8,268] flattens partition-major)
    blocks = [cslo, -cslo, sclo, -sclo]                        # per q
    cstlo16 = np.empty((16, 256))
    for up in range(4):
        for q in range(4):
            cstlo16[(8 if q >= 2 else 0) + up * 2 + (q % 2)] = blocks[q][up]

    w_v = np.ones(NV)
    w_v[1:NV - 1] = 2.0
    RC = (Cm[:, :NV] * w_v[None, :]).T                          # (67,128)
    RS = (-Sm[:, :NV] * w_v[None, :]).T
    rcsd = np.concatenate([RC, RS], axis=1)                     # (67,256)

    f16 = lambda a: np.ascontiguousarray(a.astype(np.float16))
    return {
        "khi_a": f16(khi_a), "khi_b": f16(khi_b),
        "klo_a": f16(klo_a), "klo_b": f16(klo_b),
        "gt": f16(gt), "gbv": f16(gbv),
        "csthi4": f16(csthi4), "cstlo16": f16(cstlo16), "rcsd": f16(rcsd),
    }


# --------------------------------------------------------------------------
# device kernel
# --------------------------------------------------------------------------

def build_nc(n_chan=CPC, n_batch=B, gelu=True):
    act_fn = AF.Gelu if gelu else AF.Copy
    n_img = n_chan * n_batch
    nc = bacc.Bacc("TRN2", target_bir_lowering=False, debug=False,
                   enable_asserts=False)

    x_t = nc.dram_tensor("x", [n_img, N0, N0], F16, kind="ExternalInput")
    khi_a_t = nc.dram_tensor("khi_a", [n_chan, 128, 2 * PH4], F16,
                             kind="ExternalInput")
    khi_b_t = nc.dram_tensor("khi_b", [n_chan, 128, 2 * PH4], F16,
                             kind="ExternalInput")
    klo_a_t = nc.dram_tensor("klo_a", [n_chan, 4, 2 * PH4], F16,
                             kind="ExternalInput")
    klo_b_t = nc.dram_tensor("klo_b", [n_chan, 4, 2 * PH4], F16,
                             kind="ExternalInput")
    gt_t = nc.dram_tensor("gt", [128, 2 * NP], F16, kind="ExternalInput")
    gbv_t = nc.dram_tensor("gbv", [128, 4 * NV], F16, kind="ExternalInput")
    csthi4_t = nc.dram_tensor("csthi4", [128, 1024], F16, kind="ExternalInput")
    cstlo16_t = nc.dram_tensor("cstlo16", [16, 256], F16, kind="ExternalInput")
    rcsd_t = nc.dram_tensor("rcsd", [NV, 256], F16, kind="ExternalInput")
    out_t = nc.dram_tensor("out", [n_img, 2 * N0, 2 * N0], F32,
                           kind="ExternalOutput")

    with tile.TileContext(nc) as tc:
        with (
            tc.tile_pool(name="consts", bufs=1) as cpool,
            tc.tile_pool(name="kdd", bufs=2) as kpool,
            tc.tile_pool(name="xin", bufs=4) as xpool,
            tc.tile_pool(name="r1", bufs=2) as r1pool,
            tc.tile_pool(name="ysb", bufs=2) as ypool,
            tc.tile_pool(name="fxt", bufs=2) as fxtpool,
            tc.tile_pool(name="t1", bufs=2) as t1pool,
            tc.tile_pool(name="osb", bufs=2) as opool,
            tc.tile_pool(name="ppa", bufs=2, space="PSUM") as ppa_pool,
            tc.tile_pool(name="py", bufs=2, space="PSUM") as py_pool,
            tc.tile_pool(name="pt1", bufs=2, space="PSUM") as pt1_pool,
            tc.tile_pool(name="ppd", bufs=2, space="PSUM") as ppd_pool,
        ):
            gt = cpool.tile([128, 2 * NP], F16)
            nc.sync.dma_start(gt[:], gt_t[:])
            gbv = cpool.tile([128, 4 * NV], F16)
            nc.sync.dma_start(gbv[:], gbv_t[:])
            csthi4 = cpool.tile([128, 1024], F16)
            nc.sync.dma_start(csthi4[:], csthi4_t[:])
            cstlo16 = cpool.tile([16, 256], F16)
            nc.sync.dma_start(cstlo16[:], cstlo16_t[:])
            rcsd = cpool.tile([NV, 256], F16)
            nc.sync.dma_start(rcsd[:], rcsd_t[:])

            for ci in range(n_chan):
                khi_a = kpool.tile([128, 2 * PH4], F16, tag="khi_a")
                nc.sync.dma_start(khi_a[:], khi_a_t[ci])
                khi_b = kpool.tile([128, 2 * PH4], F16, tag="khi_b")
                nc.sync.dma_start(khi_b[:], khi_b_t[ci])
                klo_a = kpool.tile([4, 2 * PH4], F16, tag="klo_a")
                nc.sync.dma_start(klo_a[:], klo_a_t[ci])
                klo_b = kpool.tile([4, 2 * PH4], F16, tag="klo_b")
                nc.sync.dma_start(klo_b[:], klo_b_t[ci])

                for bi in range(n_batch):
                    img = ci * n_batch + bi

                    # ---- stage A: r1[y, u_ri] = x^T @ gt ----
                    x_tile = xpool.tile([N0, N0], F16, tag="x")
                    nc.sync.dma_start(x_tile[:], x_t[img])
                    pA = ppa_pool.tile([128, NU], F32, tag="pA")
                    nc.tensor.matmul(pA[:], x_tile[:], gt[:],
                                     start=True, stop=True)
                    r1 = r1pool.tile([128, NU], F16, tag="r1")
                    nc.scalar.activation(r1[:], pA[:], AF.Copy)

                    # ---- stage B: Y[u, (rv)] hi + lo into one bank ----
                    pY = py_pool.tile([128, 2 * 2 * NV], F32, tag="pY")
                    yT = pY[:, 0:2 * NV]
                    ylo = pY[0:4, 2 * NV:4 * NV]
                    nc.tensor.matmul(yT, r1[:, 0:128], gbv[:, 0:2 * NV],
                                     start=True, stop=False,
                                     skip_group_check=True)
                    nc.tensor.matmul(yT, r1[:, NP:NP + 128],
                                     gbv[:, 2 * NV:4 * NV],
                                     start=False, stop=True,
                                     skip_group_check=True)
                    nc.tensor.matmul(ylo, r1[:, 128:NP], gbv[:, 0:2 * NV],
                                     start=True, stop=False,
                                     skip_group_check=True)
                    nc.tensor.matmul(ylo, r1[:, NP + 128:2 * NP],
                                     gbv[:, 2 * NV:4 * NV],
                                     start=False, stop=True,
                                     skip_group_check=True)

                    # evict Y to fp16 SBUF in ONE activation: cols 0:134
                    # hold yT (128 rows), cols 134:268 hold ylo (rows 0:4;
                    # rows 4:128 there are junk and never read)
                    ysbx = ypool.tile([128, 4 * NV], F16, tag="ysbx")
                    nc.scalar.activation(ysbx[:], pY[:], AF.Copy)
                    ysb = ysbx[:, 0:2 * NV]
                    ylo_sb = ysbx[0:4, 2 * NV:4 * NV]

                    # ---- FX products (no combines) ----
                    ybv = (ysb.rearrange("p (h v) -> p h v", h=2)
                           [:, :, None, :].broadcast_to([128, 2, 4, NV]))
                    tatb = fxtpool.tile([128, 4 * PH4], F16, tag="tatb")
                    nc.vector.tensor_mul(
                        tatb[:, 0:2 * PH4].rearrange(
                            "p (h f v) -> p h f v", h=2, f=4),
                        khi_a[:].rearrange("p (h f v) -> p h f v", h=2, f=4),
                        ybv)
                    nc.vector.tensor_mul(
                        tatb[:, 2 * PH4:4 * PH4].rearrange(
                            "p (h f v) -> p h f v", h=2, f=4),
                        khi_b[:].rearrange("p (h f v) -> p h f v", h=2, f=4),
                        ybv)

                    ylbv = (ylo_sb.rearrange("p (h v) -> p h v", h=2)
                            [:, :, None, :].broadcast_to([4, 2, 4, NV]))
                    tatbl = fxtpool.tile([4, 4 * PH4], F16, tag="tatbl")
                    nc.gpsimd.tensor_mul(
                        tatbl[:, 0:2 * PH4].rearrange(
                            "p (h f v) -> p h f v", h=2, f=4),
                        klo_a[:].rearrange("p (h f v) -> p h f v", h=2, f=4),
                        ylbv)
                    nc.gpsimd.tensor_mul(
                        tatbl[:, 2 * PH4:4 * PH4].rearrange(
                            "p (h f v) -> p h f v", h=2, f=4),
                        klo_b[:].rearrange("p (h f v) -> p h f v", h=2, f=4),
                        ylbv)
                    # reshape [4,(q,c)] -> [16,268] for one lo accum
                    # (row = 8*(q>=2) + u'*2 + q%2; cstlo16 matches)
                    tatbl16 = fxtpool.tile([16, PH4], F16, tag="tatbl16")
                    nc.sync.dma_start(tatbl16[0:8, :], tatbl[:, 0:2 * PH4])
                    nc.sync.dma_start(tatbl16[8:16, :],
                                      tatbl[:, 2 * PH4:4 * PH4])

                    # ---- stage C: 5 sign-folded accums per 256 block ----
                    t1g = t1pool.tile([NV, 1024], F16, tag="t1g")
                    for half in range(2):
                        pT1 = pt1_pool.tile([NV, 512], F32, tag="pT1")
                        for k in range(2):
                            p = 2 * half + k
                            o = pT1[:, k * 256:(k + 1) * 256]
                            for q in range(4):
                                sl = slice(q * PH4 + p * NV,
                                           q * PH4 + (p + 1) * NV)
                                nc.tensor.matmul(
                                    o, tatb[:, sl],
                                    csthi4[:, q * 256:(q + 1) * 256],
                                    start=(q == 0), stop=False)
                            nc.tensor.matmul(
                                o, tatbl16[:, p * NV:(p + 1) * NV],
                                cstlo16[:], start=False, stop=True)
                        dst = t1g[:, half * 512:(half + 1) * 512]
                        if half == 0:
                            nc.vector.tensor_copy(dst, pT1[:])
                        else:
                            nc.scalar.activation(dst, pT1[:], AF.Copy)

                    # ---- stage D: pD[x, (p, y)], 2 accums per phase ----
                    pD = ppd_pool.tile([128, 512], F32, tag="pD")
                    for p in range(4):
                        o = pD[:, p * 128:(p + 1) * 128]
                        nc.tensor.matmul(o, t1g[:, p * 256:p * 256 + 128],
                                         rcsd[:, 0:128],
                                         start=True, stop=False)
                        nc.tensor.matmul(o, t1g[:, p * 256 + 128:(p + 1) * 256],
                                         rcsd[:, 128:256],
                                         start=False, stop=True)

                    # ---- gelu + (dx, n, dy)-ordered store ----
                    outt = opool.tile([128, 512], F32, tag="outt")
                    nc.scalar.activation(
                        outt[:].rearrange("p (dx n dy) -> p dx dy n",
                                          dx=2, n=128),
                        pD[:].rearrange("p (dx dy n) -> p dx dy n",
                                        dx=2, dy=2),
                        act_fn)
                    nc.scalar.dma_start(
                        out_t[img].rearrange("(m dx) (n dy) -> m (dx n dy)",
                                             dx=2, dy=2),
                        outt[:])

    nc.compile()
    return nc


# --------------------------------------------------------------------------
# public entry point: full inputs in, full output out
# --------------------------------------------------------------------------

def kernel(x, weight, bias):
    global LAST_EXEC_NS
    x16 = np.ascontiguousarray(np.asarray(x, dtype=np.float16))
    consts = _host_constants(weight, bias)

    nc = build_nc()

    in_maps = []
    for core in range(NCORES):
        c0 = core * CPC
        xs = np.ascontiguousarray(
            x16[:, c0:c0 + CPC].transpose(1, 0, 2, 3)).reshape(NIMG, N0, N0)
        in_maps.append({
            "x": xs,
            "khi_a": np.ascontiguousarray(consts["khi_a"][c0:c0 + CPC]),
            "khi_b": np.ascontiguousarray(consts["khi_b"][c0:c0 + CPC]),
            "klo_a": np.ascontiguousarray(consts["klo_a"][c0:c0 + CPC]),
            "klo_b": np.ascontiguousarray(consts["klo_b"][c0:c0 + CPC]),
            "gt": consts["gt"],
            "gbv": consts["gbv"],
            "csthi4": consts["csthi4"],
            "cstlo16": consts["cstlo16"],
            "rcsd": consts["rcsd"],
        })

    trace = os.environ.get("KERNEL_TRACE", "0") == "1"
    tmpdir = os.environ.get("KERNEL_TMPDIR") or None
    res = run_bass_kernel_spmd(nc, in_maps, list(range(NCORES)), trace=trace,
                               tmpdir=tmpdir)
    LAST_EXEC_NS = res.exec_time_ns

    out = np.empty((B, C, 2 * N0, 2 * N0), dtype=np.float32)
    for core in range(NCORES):
        c0 = core * CPC
        o = res.results[core]["out"].reshape(CPC, B, 2 * N0, 2 * N0)
        out[:, c0:c0 + CPC] = o.transpose(1, 0, 2, 3)
    return out


# revision 17
# speedup vs baseline: 1.0795x; 1.0286x over previous
"""Converse2D-Up (FFT deconvolution upsampler) as a Bass/Tile kernel for TRN2.

v3: fp16 + FX subtracts fused into stage C's PSUM accumulation.

Measured HW facts driving this structure (see ldw_bench.py):
- PE streams 0.42 ns/col (2.4 GHz); LDWEIGHTS hides behind the previous
  matmul when stationary K>=64ish; K<=8 stationaries cost ~2x per col.
- DVE/Pool tensor ops are free-dim BYTE-bound (~0.39 ns/B) regardless of
  partition count, so [4, x] lo ops cost the same as [128, x] ones and
  every elementwise byte matters.

Per image:
  A:  r1[y, u_ri] = x^T @ gt                                (1 mm, N=264)
  B:  Y[u,(Yr v|Yi v)] PSUM, hi 128 + lo 4 rows             (4 mms, N=134)
  FX: products only (no combine):
      DVE : tatb[128,(g,h,f,v)]  = [khi_a|khi_b] (.) Y-bcast   (2 mults)
      Pool: tatbl[4,(g,h,f,v)]   = [klo_a|klo_b] (.) Ylo-bcast (2 mults)
      DMA : tatbl[4,(q,c)] -> tatbl16[16=(u',q),268]  (reshape, 16 desc)
  C:  per 256-col block [Tr|Ti]_p: 5 accums with sign-folded movings
        P1@[CT|ST] + P2@-[CT|ST] + P3@[-ST|CT] + P4'@[ST|-CT]
        + tatbl16@cstlo16                                   (20 mms, N=256)
  D:  pD[x,(p,y)] += Tr_p @ RC + Ti_p @ RS                  (8 mms, N=128)
  gelu + (dx, n, dy)-ordered store: one ACT + one DMA per image.

Sharding: 8 channels/core x 4 batch images; all spectra host-precomputed.
"""

import os

import numpy as np

import concourse.bass as bass
import concourse.mybir as mybir
import concourse.tile as tile
from concourse import bacc
from concourse.bass_utils import run_bass_kernel_spmd

F32 = mybir.dt.float32
F16 = mybir.dt.float16
AF = mybir.ActivationFunctionType

SCALE = 2
PAD = 2
EPS = 1e-5
N0 = 128           # input spatial size
NP = N0 + 2 * PAD  # 132 padded
NU = NP * SCALE    # 264 upsampled
NV = NP // 2 + 1   # 67 unique spectral columns
B = 4
C = 64
NCORES = 8
CPC = C // NCORES  # 8 channels per core
NIMG = B * CPC     # 32 images per core
PH4 = 4 * NV       # 268

LAST_EXEC_NS = None  # set by kernel() when tracing is enabled


# --------------------------------------------------------------------------
# host-side constant precompute (weight/bias -> per-channel spectra)
# --------------------------------------------------------------------------

def _host_constants(weight, bias):
    w64 = np.asarray(weight, dtype=np.float64)
    b64 = np.asarray(bias, dtype=np.float64)

    k_h, k_w = w64.shape[-2:]
    otf = np.zeros((C, NU, NU), dtype=np.complex128)
    otf[:, :k_h, :k_w] = w64[0]
    otf = np.roll(otf, (-(k_h // 2), -(k_w // 2)), axis=(-2, -1))
    FB = np.fft.fftn(otf, axes=(-2, -1))                      # (C,264,264)

    biaseps = 1.0 / (1.0 + np.exp(-(b64.reshape(C) - 9.0))) + EPS
    be = biaseps[:, None, None]

    u = np.arange(NU)
    Dr = 1 + np.exp(-2j * np.pi * u / NU)
    D = Dr[:, None] * Dr[None, :]

    Gh = np.conj(FB) + be * D[None]
    FBG = FB * Gh

    def quadmean(A):
        return 0.25 * (A[:, :NP, :NP] + A[:, NP:, :NP]
                       + A[:, :NP, NP:] + A[:, NP:, NP:])

    M1 = quadmean(FBG)
    invW = quadmean(np.abs(FB) ** 2)
    M2 = M1 / (invW + be)
    H = (Gh - np.conj(FB) * np.tile(M2, (1, SCALE, SCALE))) / be

    hr = np.fft.ifft2(H, axes=(-2, -1)).real
    kdd = np.empty((C, 2, 2, NP, NV), dtype=np.complex128)
    for dx in range(2):
        for dy in range(2):
            kh = np.fft.fft2(hr[:, dx::2, dy::2], axes=(-2, -1))
            kdd[:, dx, dy] = kh[:, :, :NV]

    # K tiles: [u, (f, v)] per real/imag, phases f = 2*dx+dy
    kr = np.empty((C, NP, PH4), dtype=np.float64)
    ki = np.empty((C, NP, PH4), dtype=np.float64)
    for dx in range(2):
        for dy in range(2):
            p = dx * 2 + dy
            kr[:, :, p * NV:(p + 1) * NV] = kdd[:, dx, dy].real
            ki[:, :, p * NV:(p + 1) * NV] = kdd[:, dx, dy].imag

    # khi_a = [Kr | Ki], khi_b = [Ki | -Kr].
    # Products: P1=KrYr P2=KiYi P3=KiYr P4'=-KrYi  (q=(g,h) in order)
    khi_a = np.concatenate([kr[:, :128], ki[:, :128]], axis=2)
    khi_b = np.concatenate([ki[:, :128], -kr[:, :128]], axis=2)
    klo_a = np.concatenate([kr[:, 128:], ki[:, 128:]], axis=2)
    klo_b = np.concatenate([ki[:, 128:], -kr[:, 128:]], axis=2)

    # forward matrix G = F132 @ P  (132x128 complex)
    P = np.zeros((NP, N0))
    for m in range(NP):
        P[m, (m - PAD) % N0] = 1.0
    F132 = np.exp(-2j * np.pi * np.outer(np.arange(NP), np.arange(NP)) / NP)
    G = F132 @ P
    gt = np.concatenate([G.real.T, G.imag.T], axis=1)          # (128, 264)

    # stage-B movings: [Gr | Gi | -Gi | Gr]  (128, 268)
    gbv = np.concatenate([gt[:, 0:NV], gt[:, NP:NP + NV],
                          -gt[:, NP:NP + NV], gt[:, 0:NV]], axis=1)

    # inverse rows i in [2,130) of iF132/132
    Ai = np.exp(2j * np.pi * np.outer(np.arange(2, 130), np.arange(NP)) / NP) / NP
    Cm, Sm = Ai.real, Ai.imag
    CT = Cm.T                                                  # (132,128)
    ST = Sm.T

    # C movings, sign-folded per product q:
    #   [Tr|Ti] = P1@[CT|ST] - P2@[CT|ST] + P3@[-ST|CT] - P4'@[-ST|CT]
    cs = np.concatenate([CT[:128], ST[:128]], axis=1)          # (128,256)
    sc = np.concatenate([-ST[:128], CT[:128]], axis=1)
    csthi4 = np.concatenate([cs, -cs, sc, -sc], axis=1)        # (128,1024)

    # lo: tatbl16 rows (u', q); cstlo16 row (u', q) = sign_q * block_q(u')
    cslo = np.concatenate([CT[128:], ST[128:]], axis=1)        # (4,256)
    sclo = np.concatenate([-ST[128:], CT[128:]], axis=1)
    # rows match the 2-DMA reshape: row = 8*(q>=2) + u'*2 + (q%2)
    # (HW-verified: dma [4,536]->[8,268] flattens partition-major)
    blocks = [cslo, -cslo, sclo, -sclo]                        # per q
    cstlo16 = np.empty((16, 256))
    for up in range(4):
        for q in range(4):
            cstlo16[(8 if q >= 2 else 0) + up * 2 + (q % 2)] = blocks[q][up]

    w_v = np.ones(NV)
    w_v[1:NV - 1] = 2.0
    RC = (Cm[:, :NV] * w_v[None, :]).T                          # (67,128)
    RS = (-Sm[:, :NV] * w_v[None, :]).T
    rcsd = np.concatenate([RC, RS], axis=1)                     # (67,256)

    f16 = lambda a: np.ascontiguousarray(a.astype(np.float16))
    return {
        "khi_a": f16(khi_a), "khi_b": f16(khi_b),
        "klo_a": f16(klo_a), "klo_b": f16(klo_b),
        "gt": f16(gt), "gbv": f16(gbv),
        "csthi4": f16(csthi4), "cstlo16": f16(cstlo16), "rcsd": f16(rcsd),
    }


# --------------------------------------------------------------------------
# device kernel
# --------------------------------------------------------------------------

def build_nc(n_chan=CPC, n_batch=B, gelu=True):
    act_fn = AF.Gelu if gelu else AF.Copy
    n_img = n_chan * n_batch
    nc = bacc.Bacc("TRN2", target_bir_lowering=False, debug=False,
                   enable_asserts=False)

    x_t = nc.dram_tensor("x", [n_img, N0, N0], F16, kind="ExternalInput")
    khi_a_t = nc.dram_tensor("khi_a", [n_chan, 128, 2 * PH4], F16,
                             kind="ExternalInput")
    khi_b_t = nc.dram_tensor("khi_b", [n_chan, 128, 2 * PH4], F16,
                             kind="ExternalInput")
    klo_a_t = nc.dram_tensor("klo_a", [n_chan, 4, 2 * PH4], F16,
                             kind="ExternalInput")
    klo_b_t = nc.dram_tensor("klo_b", [n_chan, 4, 2 * PH4], F16,
                             kind="ExternalInput")
    gt_t = nc.dram_tensor("gt", [128, 2 * NP], F16, kind="ExternalInput")
    gbv_t = nc.dram_tensor("gbv", [128, 4 * NV], F16, kind="ExternalInput")
    csthi4_t = nc.dram_tensor("csthi4", [128, 1024], F16, kind="ExternalInput")
    cstlo16_t = nc.dram_tensor("cstlo16", [16, 256], F16, kind="ExternalInput")
    rcsd_t = nc.dram_tensor("rcsd", [NV, 256], F16, kind="ExternalInput")
    out_t = nc.dram_tensor("out", [n_img, 2 * N0, 2 * N0], F32,
                           kind="ExternalOutput")

    with tile.TileContext(nc) as tc:
        with (
            tc.tile_pool(name="consts", bufs=1) as cpool,
            tc.tile_pool(name="kdd", bufs=2) as kpool,
            tc.tile_pool(name="xin", bufs=4) as xpool,
            tc.tile_pool(name="r1", bufs=2) as r1pool,
            tc.tile_pool(name="ysb", bufs=2) as ypool,
            tc.tile_pool(name="fxt", bufs=2) as fxtpool,
            tc.tile_pool(name="t1", bufs=2) as t1pool,
            tc.tile_pool(name="osb", bufs=2) as opool,
            tc.tile_pool(name="ppa", bufs=2, space="PSUM") as ppa_pool,
            tc.tile_pool(name="py", bufs=2, space="PSUM") as py_pool,
            tc.tile_pool(name="pt1", bufs=2, space="PSUM") as pt1_pool,
            tc.tile_pool(name="ppd", bufs=2, space="PSUM") as ppd_pool,
        ):
            gt = cpool.tile([128, 2 * NP], F16)
            nc.sync.dma_start(gt[:], gt_t[:])
            gbv = cpool.tile([128, 4 * NV], F16)
            nc.sync.dma_start(gbv[:], gbv_t[:])
            csthi4 = cpool.tile([128, 1024], F16)
            nc.sync.dma_start(csthi4[:], csthi4_t[:])
            cstlo16 = cpool.tile([16, 256], F16)
            nc.sync.dma_start(cstlo16[:], cstlo16_t[:])
            rcsd = cpool.tile([NV, 256], F16)
            nc.sync.dma_start(rcsd[:], rcsd_t[:])

            for ci in range(n_chan):
                khi_a = kpool.tile([128, 2 * PH4], F16, tag="khi_a")
                nc.sync.dma_start(khi_a[:], khi_a_t[ci])
                khi_b = kpool.tile([128, 2 * PH4], F16, tag="khi_b")
                nc.sync.dma_start(khi_b[:], khi_b_t[ci])
                klo_a = kpool.tile([4, 2 * PH4], F16, tag="klo_a")
                nc.sync.dma_start(klo_a[:], klo_a_t[ci])
                klo_b = kpool.tile([4, 2 * PH4], F16, tag="klo_b")
                nc.sync.dma_start(klo_b[:], klo_b_t[ci])

                for bi in range(n_batch):
                    img = ci * n_batch + bi

                    # ---- stage A: r1[y, u_ri] = x^T @ gt ----
                    x_tile = xpool.tile([N0, N0], F16, tag="x")
                    nc.sync.dma_start(x_tile[:], x_t[img])
                    pA = ppa_pool.tile([128, NU], F32, tag="pA")
                    nc.tensor.matmul(pA[:], x_tile[:], gt[:],
                                     start=True, stop=True)
                    r1 = r1pool.tile([128, NU], F16, tag="r1")
                    nc.scalar.activation(r1[:], pA[:], AF.Copy)

                    # ---- stage B: Y[u, (rv)] hi + lo into one bank ----
                    pY = py_pool.tile([128, 2 * 2 * NV], F32, tag="pY")
                    yT = pY[:, 0:2 * NV]
                    ylo = pY[0:4, 2 * NV:4 * NV]
                    nc.tensor.matmul(yT, r1[:, 0:128], gbv[:, 0:2 * NV],
                                     start=True, stop=False,
                                     skip_group_check=True)
                    nc.tensor.matmul(yT, r1[:, NP:NP + 128],
                                     gbv[:, 2 * NV:4 * NV],
                                     start=False, stop=True,
                                     skip_group_check=True)
                    nc.tensor.matmul(ylo, r1[:, 128:NP], gbv[:, 0:2 * NV],
                                     start=True, stop=False,
                                     skip_group_check=True)
                    nc.tensor.matmul(ylo, r1[:, NP + 128:2 * NP],
                                     gbv[:, 2 * NV:4 * NV],
                                     start=False, stop=True,
                                     skip_group_check=True)

                    # evict Y to fp16 SBUF
                    ysb = ypool.tile([128, 2 * NV], F16, tag="ysb")
                    nc.scalar.activation(ysb[:], yT, AF.Copy)
                    ylo_sb = ypool.tile([4, 2 * NV], F16, tag="ylo_sb")
                    nc.scalar.activation(ylo_sb[:], ylo, AF.Copy)

                    # ---- FX products (no combines) ----
                    ybv = (ysb[:].rearrange("p (h v) -> p h v", h=2)
                           [:, :, None, :].broadcast_to([128, 2, 4, NV]))
                    tatb = fxtpool.tile([128, 4 * PH4], F16, tag="tatb")
                    nc.vector.tensor_mul(
                        tatb[:, 0:2 * PH4].rearrange(
                            "p (h f v) -> p h f v", h=2, f=4),
                        khi_a[:].rearrange("p (h f v) -> p h f v", h=2, f=4),
                        ybv)
                    nc.vector.tensor_mul(
                        tatb[:, 2 * PH4:4 * PH4].rearrange(
                            "p (h f v) -> p h f v", h=2, f=4),
                        khi_b[:].rearrange("p (h f v) -> p h f v", h=2, f=4),
                        ybv)

                    ylbv = (ylo_sb[:].rearrange("p (h v) -> p h v", h=2)
                            [:, :, None, :].broadcast_to([4, 2, 4, NV]))
                    tatbl = fxtpool.tile([4, 4 * PH4], F16, tag="tatbl")
                    nc.gpsimd.tensor_mul(
                        tatbl[:, 0:2 * PH4].rearrange(
                            "p (h f v) -> p h f v", h=2, f=4),
                        klo_a[:].rearrange("p (h f v) -> p h f v", h=2, f=4),
                        ylbv)
                    nc.gpsimd.tensor_mul(
                        tatbl[:, 2 * PH4:4 * PH4].rearrange(
                            "p (h f v) -> p h f v", h=2, f=4),
                        klo_b[:].rearrange("p (h f v) -> p h f v", h=2, f=4),
                        ylbv)
                    # reshape [4,(q,c)] -> [16,268] for one lo accum
                    # (row = 8*(q>=2) + u'*2 + q%2; cstlo16 matches)
                    tatbl16 = fxtpool.tile([16, PH4], F16, tag="tatbl16")
                    nc.sync.dma_start(tatbl16[0:8, :], tatbl[:, 0:2 * PH4])
                    nc.sync.dma_start(tatbl16[8:16, :],
                                      tatbl[:, 2 * PH4:4 * PH4])

                    # ---- stage C: 5 sign-folded accums per 256 block ----
                    t1g = t1pool.tile([NV, 1024], F16, tag="t1g")
                    for half in range(2):
                        pT1 = pt1_pool.tile([NV, 512], F32, tag="pT1")
                        for k in range(2):
                            p = 2 * half + k
                            o = pT1[:, k * 256:(k + 1) * 256]
                            for q in range(4):
                                sl = slice(q * PH4 + p * NV,
                                           q * PH4 + (p + 1) * NV)
                                nc.tensor.matmul(
                                    o, tatb[:, sl],
                                    csthi4[:, q * 256:(q + 1) * 256],
                                    start=(q == 0), stop=False)
                            nc.tensor.matmul(
                                o, tatbl16[:, p * NV:(p + 1) * NV],
                                cstlo16[:], start=False, stop=True)
                        dst = t1g[:, half * 512:(half + 1) * 512]
                        if half == 0:
                            nc.vector.tensor_copy(dst, pT1[:])
                        else:
                            nc.scalar.activation(dst, pT1[:], AF.Copy)

                    # ---- stage D: pD[x, (p, y)], 2 accums per phase ----
                    pD = ppd_pool.tile([128, 512], F32, tag="pD")
                    for p in range(4):
                        o = pD[:, p * 128:(p + 1) * 128]
                        nc.tensor.matmul(o, t1g[:, p * 256:p * 256 + 128],
                                         rcsd[:, 0:128],
                                         start=True, stop=False)
                        nc.tensor.matmul(o, t1g[:, p * 256 + 128:(p + 1) * 256],
                                         rcsd[:, 128:256],
                                         start=False, stop=True)

                    # ---- gelu + (dx, n, dy)-ordered store ----
                    outt = opool.tile([128, 512], F32, tag="outt")
                    nc.scalar.activation(
                        outt[:].rearrange("p (dx n dy) -> p dx dy n",
                                          dx=2, n=128),
                        pD[:].rearrange("p (dx dy n) -> p dx dy n",
                                        dx=2, dy=2),
                        act_fn)
                    nc.scalar.dma_start(
                        out_t[img].rearrange("(m dx) (n dy) -> m (dx n dy)",
                                             dx=2, dy=2),
                        outt[:])

    nc.compile()
    return nc


# --------------------------------------------------------------------------
# public entry point: full inputs in, full output out
# --------------------------------------------------------------------------

def kernel(x, weight, bias):
    global LAST_EXEC_NS
    x16 = np.ascontiguousarray(np.asarray(x, dtype=np.float16))
    consts = _host_constants(weight, bias)

    nc = build_nc()

    in_maps = []
    for core in range(NCORES):
        c0 = core * CPC
        xs = np.ascontiguousarray(
            x16[:, c0:c0 + CPC].transpose(1, 0, 2, 3)).reshape(NIMG, N0, N0)
        in_maps.append({
            "x": xs,
            "khi_a": np.ascontiguousarray(consts["khi_a"][c0:c0 + CPC]),
            "khi_b": np.ascontiguousarray(consts["khi_b"][c0:c0 + CPC]),
            "klo_a": np.ascontiguousarray(consts["klo_a"][c0:c0 + CPC]),
            "klo_b": np.ascontiguousarray(consts["klo_b"][c0:c0 + CPC]),
            "gt": consts["gt"],
            "gbv": consts["gbv"],
            "csthi4": consts["csthi4"],
            "cstlo16": consts["cstlo16"],
            "rcsd": consts["rcsd"],
        })

    trace = os.environ.get("KERNEL_TRACE", "0") == "1"
    tmpdir = os.environ.get("KERNEL_TMPDIR") or None
    res = run_bass_kernel_spmd(nc, in_maps, list(range(NCORES)), trace=trace,
                               tmpdir=tmpdir)
    LAST_EXEC_NS = res.exec_time_ns

    out = np.empty((B, C, 2 * N0, 2 * N0), dtype=np.float32)
    for core in range(NCORES):
        c0 = core * CPC
        o = res.results[core]["out"].reshape(CPC, B, 2 * N0, 2 * N0)
        out[:, c0:c0 + CPC] = o.transpose(1, 0, 2, 3)
    return out
